# revision 19
# baseline (speedup 1.0000x reference)
"""Trainium2 Bass kernel for nn_AutoEncoder (PointNet++-style encoder/decoder).

Strategy (pure data parallel, B=64 clouds over 8 cores, 8 clouds/core):
  - Host (numpy): FPS sampling + ball-query + neighbor grouping — these are
    pure index functions of the input xyz and sequential/control-flow heavy.
  - Device kernel A: SA1 pointwise MLP (6->64->64) + max-pool over the K1
    group slots, two clouds batched per matmul (K=12/128, M=128), per core.
  - Host: gather SA1 features into SA2 groups (indices precomputed).
  - Device kernel B: SA2 MLP (67->128->128) + max over K2, SA3 global
    MLP (131->256->256) + max, and the FC head (256->512->512->768) with
    GroupNorm(1, C) — all per core on 8 clouds.

Key exact optimizations:
  - BatchNorm (eval) folds into relu(s*(W@x)+t); s>0 lets scale/bias/relu of
    each block's last layer commute past the max-pool.
  - Group padding slots are duplicates of a real member, and every layer is
    pointwise before a max — so groups can be truncated to the actual max
    in-radius count (K1/K2 measured on the host, kernels compiled per size).
  - Matmuls use float32r (fp32 data, fast PE mode).
"""

import numpy as np

import concourse.bass as bass
import concourse.bacc as bacc
import concourse.tile as tile
from concourse import mybir
from concourse.bass_utils import run_bass_kernel_spmd

F32 = mybir.dt.float32
F32R = mybir.dt.float32r
AF = mybir.ActivationFunctionType
AX = mybir.AxisListType
OP = mybir.AluOpType

EPS = 1e-5
INV = np.float32(1.0 / np.sqrt(1.0 + EPS))
NCORES = 8
CPC = 8  # clouds per core
NPAIR = 4  # cloud pairs per core (SA1 batches 2 clouds per matmul)

# packed const column offsets, kernel B "wb" [128, WB_COLS]
WB_W2A = 0
WB_W2B = 128
WB_W3AX = 256
WB_W3AP = 512
WB_W3B = 768        # [128, 2, 256]
WB_WFC1 = 1280      # [128, 2, 512]
WB_WFC2 = 2304      # [128, 4, 512]
WB_WFC3 = 4352      # [128, 4, 768]
WB_SC = 7424        # 12 cols: s2a,t2a,s2b,t2b,s3a0,s3a1,t3a0,t3a1,s3b0,s3b1,t3b0,t3b1
WB_COLS = 7436

# packed fc row-const offsets, kernel B "fcb" [8, FCB_COLS]
FCB_BFC1 = 0
FCB_GN1G = 512
FCB_GN1B = 1024
FCB_BFC2 = 1536
FCB_GN2G = 2048
FCB_GN2B = 2560
FCB_BFC3 = 3072
FCB_ID8 = 3840
FCB_COLS = 3848


# ---------------------------------------------------------------------------
# Host-side index math (pure functions of input xyz)
# ---------------------------------------------------------------------------


def _fps(pts, npoint):
    B, N, _ = pts.shape
    dist = np.full((B, N), 1e10, np.float32)
    far = np.zeros(B, np.int64)
    idx = np.empty((B, npoint), np.int32)
    ar = np.arange(B)
    for i in range(npoint):
        idx[:, i] = far
        c = pts[ar, far]
        d = ((pts - c[:, None, :]) ** 2).sum(-1, dtype=np.float32)
        dist = np.minimum(dist, d)
        far = dist.argmax(-1)
    return idx


def _ball_query(radius, nsample, xyz, new_xyz):
    B, N, _ = xyz.shape
    sqr = (
        (new_xyz * new_xyz).sum(-1, dtype=np.float32)[:, :, None]
        + (xyz * xyz).sum(-1, dtype=np.float32)[:, None, :]
        - np.float32(2.0) * np.einsum("bsc,bnc->bsn", new_xyz, xyz).astype(np.float32)
    )
    inr = sqr <= np.float32(radius * radius)
    cnt = inr.sum(-1)
    idx = np.where(inr, np.arange(N, dtype=np.int32), N).astype(np.int32)
    part = np.partition(idx, nsample - 1, axis=-1)[:, :, :nsample]
    part = np.sort(part, axis=-1)
    first = part[:, :, :1]
    return np.where(part == N, first, part), int(cnt.max())


def _fold_conv(layer):
    # (W,b,g,bt): layer(x) == relu(s*(W@x) + t)
    W, b, g, bt = [np.asarray(a, np.float32) for a in layer]
    s = (g * INV).astype(np.float32)
    t = (s * b + bt).astype(np.float32)
    assert (s > 0).all(), "max/scale commute needs s>0"
    return np.ascontiguousarray(W), s, t


# ---------------------------------------------------------------------------
# Bass kernel A: SA1 (6 -> 64 -> 64, max over K1) for 4 cloud-pairs
# ---------------------------------------------------------------------------


def build_kernel_a(k1, relu_split=False):
    slots = 256 * k1          # group slots per cloud
    csz = min(512, slots)     # matmul chunk width
    nch = (slots + csz - 1) // csz
    spc = csz // k1           # centers per chunk

    nc = bacc.Bacc()
    g1 = nc.dram_tensor("g1", [NPAIR, 12, slots], F32R, kind="ExternalInput")
    # packed consts: cols 0:128 w1 (rows 0:12), 128:256 w2, 256 s1, 257 t1, 258 s2, 259 t2
    wa = nc.dram_tensor("wa", [128, 260], F32R, kind="ExternalInput")
    l1out = nc.dram_tensor("l1out", [NPAIR, 128, 256], F32, kind="ExternalOutput")

    with tile.TileContext(nc) as tc:
        with (
            tc.tile_pool(name="consts", bufs=1) as consts,
            tc.tile_pool(name="gin", bufs=2) as gin,
            tc.tile_pool(name="hbuf", bufs=2) as hbuf,
            tc.tile_pool(name="obuf", bufs=2) as obuf,
            tc.tile_pool(name="ps", bufs=4, space="PSUM") as ps,
        ):
            wt = consts.tile([128, 260], F32R)
            nc.sync.dma_start(out=wt, in_=wa[:])
            w1t = wt[0:12, 0:128]
            w2t = wt[:, 128:256]
            s1t, t1t = wt[:, 256:257].bitcast(F32), wt[:, 257:258].bitcast(F32)
            s2t, t2t = wt[:, 258:259].bitcast(F32), wt[:, 259:260].bitcast(F32)

            bigw = min(1024, slots)       # psum supertile: 2 banks
            nbig = (slots + bigw - 1) // bigw
            for p in range(NPAIR):
                g = gin.tile([12, slots], F32R)
                nc.sync.dma_start(out=g, in_=g1[p])
                h1 = hbuf.tile([128, slots], F32R)
                m2 = obuf.tile([128, 256], F32, tag="m2")
                for b in range(nbig):
                    pt = ps.tile([128, bigw], F32, tag="mm")
                    for q in range(bigw // csz):
                        qs = slice(q * csz, (q + 1) * csz)
                        gs = slice(b * bigw + q * csz, b * bigw + (q + 1) * csz)
                        nc.tensor.matmul(pt[:, qs], (w1t), (g[:, gs]), start=True, stop=True)
                    bs = slice(b * bigw, (b + 1) * bigw)
                    nc.scalar.activation(h1[:, bs], pt, AF.Relu, bias=t1t, scale=s1t)
                for b in range(nbig):
                    pt2 = ps.tile([128, bigw], F32, tag="mm")
                    for q in range(bigw // csz):
                        qs = slice(q * csz, (q + 1) * csz)
                        hs = slice(b * bigw + q * csz, b * bigw + (q + 1) * csz)
                        nc.tensor.matmul(pt2[:, qs], (w2t), (h1[:, hs]), start=True, stop=True)
                    spb = bigw // k1
                    nc.vector.tensor_reduce(
                        m2[:, b * spb:(b + 1) * spb],
                        pt2.rearrange("p (s k) -> p s k", k=k1),
                        axis=AX.X,
                        op=OP.max,
                    )
                o = obuf.tile([128, 256], F32, tag="o")
                nc.scalar.activation(o, m2, AF.Relu, bias=t2t, scale=s2t)
                nc.sync.dma_start(out=l1out[p], in_=o)
    nc.compile()
    return nc


# ---------------------------------------------------------------------------
# Bass kernel B: SA2 (67 -> 128 -> 128, max over K2) + SA3 + FC head, 8 clouds
# ---------------------------------------------------------------------------


def build_kernel_b(k2, simple_head):
    slots = 128 * k2
    csz = min(512, slots)
    nch = (slots + csz - 1) // csz
    spc = csz // k2

    nc = bacc.Bacc()
    if k2 == 1:
        g2 = nc.dram_tensor("g2", [67, CPC * 128], F32R, kind="ExternalInput")
        new2t = nc.dram_tensor("new2t", [3, CPC * 128], F32R, kind="ExternalInput")
    else:
        g2 = nc.dram_tensor("g2", [CPC, 67, slots], F32R, kind="ExternalInput")
        new2t = nc.dram_tensor("new2t", [CPC, 3, 128], F32R, kind="ExternalInput")
    wb = nc.dram_tensor("wb", [128, WB_COLS], F32R, kind="ExternalInput")
    fcb = nc.dram_tensor("fcb", [CPC, FCB_COLS], F32, kind="ExternalInput")
    out = nc.dram_tensor("out", [CPC, 768], F32, kind="ExternalOutput")

    with tile.TileContext(nc) as tc:
        with (
            tc.tile_pool(name="consts", bufs=1) as consts,
            tc.tile_pool(name="gin", bufs=2) as gin,
            tc.tile_pool(name="hbuf", bufs=2) as hbuf,
            tc.tile_pool(name="small", bufs=3) as small,
            tc.tile_pool(name="fc", bufs=2) as fcp,
            tc.tile_pool(name="psA", bufs=3, space="PSUM") as psA,
            tc.tile_pool(name="psB", bufs=2, space="PSUM") as psB,
        ):
            # SA weights+scales load first (small); FC weights stream behind
            wt = consts.tile([128, 1292], F32R, tag="wb")
            nc.sync.dma_start(out=wt[:, 0:1280], in_=wb[:, 0:1280])
            nc.sync.dma_start(out=wt[:, 1280:1292], in_=wb[:, WB_SC:WB_SC + 12])
            wf1 = consts.tile([128, 1024], F32R, tag="wf1")
            nc.sync.dma_start(out=wf1, in_=wb[:, WB_WFC1:WB_WFC1 + 1024])
            wf2 = consts.tile([128, 2048], F32R, tag="wf2")
            nc.sync.dma_start(out=wf2, in_=wb[:, WB_WFC2:WB_WFC2 + 2048])
            wf3 = consts.tile([128, 3072], F32R, tag="wf3")
            nc.sync.dma_start(out=wf3, in_=wb[:, WB_WFC3:WB_WFC3 + 3072])
            fct = consts.tile([CPC, FCB_COLS], F32, tag="fcb")
            nc.sync.dma_start(out=fct, in_=fcb[:])

            w2at = wt[0:67, WB_W2A:WB_W2A + 128]
            w2bt = wt[:, WB_W2B:WB_W2B + 128]
            w3axt = wt[0:3, WB_W3AX:WB_W3AX + 256]
            w3apt = wt[:, WB_W3AP:WB_W3AP + 256]
            w3bt = wt[:, WB_W3B:WB_W3B + 512].rearrange("p (k m) -> p k m", k=2)
            wfc1t = wf1.rearrange("p (k m) -> p k m", k=2)
            wfc2t = wf2.rearrange("p (k m) -> p k m", k=4)
            wfc3t = wf3.rearrange("p (k m) -> p k m", k=4)
            sc = 1280
            s2at, t2at = wt[:, sc + 0:sc + 1].bitcast(F32), wt[:, sc + 1:sc + 2].bitcast(F32)
            s2bt, t2bt = wt[:, sc + 2:sc + 3].bitcast(F32), wt[:, sc + 3:sc + 4].bitcast(F32)
            s3at = wt[:, sc + 4:sc + 6].bitcast(F32)
            t3at = wt[:, sc + 6:sc + 8].bitcast(F32)
            s3bt = wt[:, sc + 8:sc + 10].bitcast(F32)
            t3bt = wt[:, sc + 10:sc + 12].bitcast(F32)

            bfc1t = fct[:, FCB_BFC1:FCB_BFC1 + 512]
            gn1gt = fct[:, FCB_GN1G:FCB_GN1G + 512]
            gn1bt = fct[:, FCB_GN1B:FCB_GN1B + 512]
            bfc2t = fct[:, FCB_BFC2:FCB_BFC2 + 512]
            gn2gt = fct[:, FCB_GN2G:FCB_GN2G + 512]
            gn2bt = fct[:, FCB_GN2B:FCB_GN2B + 512]
            bfc3t = fct[:, FCB_BFC3:FCB_BFC3 + 768]
            id8t = fct[0:8, FCB_ID8:FCB_ID8 + 8]

            epst = consts.tile([CPC, 1], F32, tag="eps")
            nc.vector.memset(epst, EPS)

            l3raw = consts.tile([128, 2, CPC], F32, tag="l3raw")

            if k2 == 1:
                # All 8 clouds batched along the free dim (1024 cols).
                cols = CPC * 128
                g = gin.tile([67, cols], F32R)
                nc.sync.dma_start(out=g, in_=g2[:])
                x2 = small.tile([3, cols], F32R, tag="x2")
                nc.sync.dma_start(out=x2, in_=new2t[:])
                l2p = small.tile([128, cols], F32R, tag="l2p")
                h1s = small.tile([128, cols], F32R, tag="h1s")
                for b in range(cols // 1024):
                    bsl = slice(b * 1024, (b + 1) * 1024)
                    p1 = psA.tile([128, 1024], F32, tag="mm")
                    for q in range(2):
                        qs = slice(q * 512, (q + 1) * 512)
                        gs = slice(b * 1024 + q * 512, b * 1024 + (q + 1) * 512)
                        nc.tensor.matmul(p1[:, qs], (w2at), (g[:, gs]), start=True, stop=True)
                    nc.scalar.activation(h1s[:, bsl], p1, AF.Relu, bias=t2at, scale=s2at)
                    p2 = psA.tile([128, 1024], F32, tag="mm")
                    for q in range(2):
                        qs = slice(q * 512, (q + 1) * 512)
                        hs = slice(b * 1024 + q * 512, b * 1024 + (q + 1) * 512)
                        nc.tensor.matmul(p2[:, qs], (w2bt), (h1s[:, hs]), start=True, stop=True)
                    nc.scalar.activation(l2p[:, bsl], p2, AF.Relu, bias=t2bt, scale=s2bt)
                # SA3 on all clouds at once
                h3 = small.tile([128, 2, cols], F32R, tag="h3")
                for mm in range(2):
                    msl = slice(mm * 128, (mm + 1) * 128)
                    for b in range(cols // 1024):
                        p3 = psA.tile([128, 1024], F32, tag="mm")
                        for q in range(2):
                            qs = slice(q * 512, (q + 1) * 512)
                            cs = slice(b * 1024 + q * 512, b * 1024 + (q + 1) * 512)
                            nc.tensor.matmul(p3[:, qs], (w3axt[:, msl]), (x2[:, cs]), start=True, stop=False)
                            nc.tensor.matmul(p3[:, qs], (w3apt[:, msl]), (l2p[:, cs]), start=False, stop=True)
                        nc.scalar.activation(
                            h3[:, mm, b * 1024:(b + 1) * 1024], p3, AF.Relu,
                            bias=t3at[:, mm:mm + 1], scale=s3at[:, mm:mm + 1],
                        )
                for mm in range(2):
                    msl = slice(mm * 128, (mm + 1) * 128)
                    for b in range(cols // 1024):
                        p4 = psA.tile([128, 1024], F32, tag="mm")
                        for q in range(2):
                            qs = slice(q * 512, (q + 1) * 512)
                            cs = slice(b * 1024 + q * 512, b * 1024 + (q + 1) * 512)
                            nc.tensor.matmul(p4[:, qs], (w3bt[:, 0, msl]), (h3[:, 0, cs]), start=True, stop=False)
                            nc.tensor.matmul(p4[:, qs], (w3bt[:, 1, msl]), (h3[:, 1, cs]), start=False, stop=True)
                        nc.vector.tensor_reduce(
                            l3raw[:, mm, b * 8:(b + 1) * 8],
                            p4.rearrange("p (c n) -> p c n", n=128),
                            axis=AX.X,
                            op=OP.max,
                        )
            else:
                for c in range(CPC):
                    g = gin.tile([67, slots], F32R)
                    nc.sync.dma_start(out=g, in_=g2[c])
                    l2p = small.tile([128, 128], F32R, tag="l2p")
                    h1 = hbuf.tile([128, slots], F32R)
                    m = small.tile([128, 128], F32, tag="m")
                    for fc in range(nch):
                        sl = slice(fc * csz, (fc + 1) * csz)
                        pt = psA.tile([128, csz], F32, tag="mm")
                        nc.tensor.matmul(pt, (w2at), (g[:, sl]), start=True, stop=True)
                        nc.scalar.activation(h1[:, sl], pt, AF.Relu, bias=t2at, scale=s2at)
                    for fc in range(nch):
                        sl = slice(fc * csz, (fc + 1) * csz)
                        pt2 = psA.tile([128, csz], F32, tag="mm")
                        nc.tensor.matmul(pt2, (w2bt), (h1[:, sl]), start=True, stop=True)
                        nc.vector.tensor_reduce(
                            m[:, fc * spc:(fc + 1) * spc],
                            pt2.rearrange("p (s k) -> p s k", k=k2),
                            axis=AX.X,
                            op=OP.max,
                        )
                    nc.scalar.activation(l2p, m, AF.Relu, bias=t2bt, scale=s2bt)

                    # SA3: 131 -> 256 (relu) -> 256, max over the 128 points
                    x2 = small.tile([3, 128], F32R, tag="x2")
                    nc.sync.dma_start(out=x2, in_=new2t[c])
                    h3 = small.tile([128, 2, 128], F32R, tag="h3")
                    for mm in range(2):
                        msl = slice(mm * 128, (mm + 1) * 128)
                        p3 = psB.tile([128, 128], F32, tag="mix")
                        nc.tensor.matmul(p3, (w3axt[:, msl]), (x2), start=True, stop=False)
                        nc.tensor.matmul(p3, (w3apt[:, msl]), (l2p), start=False, stop=True)
                        nc.scalar.activation(
                            h3[:, mm, :], p3, AF.Relu,
                            bias=t3at[:, mm:mm + 1], scale=s3at[:, mm:mm + 1],
                        )
                    for mm in range(2):
                        msl = slice(mm * 128, (mm + 1) * 128)
                        p4 = psB.tile([128, 128], F32, tag="mix")
                        nc.tensor.matmul(p4, (w3bt[:, 0, msl]), (h3[:, 0, :]), start=True, stop=False)
                        nc.tensor.matmul(p4, (w3bt[:, 1, msl]), (h3[:, 1, :]), start=False, stop=True)
                        nc.vector.tensor_reduce(
                            l3raw[:, mm, c:c + 1], p4, axis=AX.X, op=OP.max
                        )

            # post-max scale/bias/relu for SA3 output: [128, 2, 8]
            l3r = fcp.tile([128, 2, CPC], F32R, tag="l3r")
            for mm in range(2):
                nc.scalar.activation(
                    l3r[:, mm, :], l3raw[:, mm, :], AF.Relu,
                    bias=t3bt[:, mm:mm + 1], scale=s3bt[:, mm:mm + 1],
                )

            def groupnorm_block(z_ps, badd, gg, bb, n_ch):
                # z_ps: PSUM [8, n_ch]; returns sbuf tile [8, n_ch] = GN(z)
                # (relu is fused into the post-transpose copy)
                if simple_head:
                    z = z_ps  # bias is zero; stats straight from PSUM
                else:
                    z = fcp.tile([CPC, n_ch], F32, tag="z")
                    nc.vector.tensor_tensor(z, z_ps, badd, OP.add)
                stats = fcp.tile([CPC, 6], F32, tag="stats")
                nc.vector.bn_stats(out=stats, in_=z)
                mv = fcp.tile([CPC, 2], F32, tag="mv")
                nc.vector.bn_aggr(out=mv, in_=stats)
                std = fcp.tile([CPC, 1], F32, tag="std")
                nc.scalar.activation(std, mv[:, 1:2], AF.Sqrt, bias=epst, scale=1.0)
                rstd = fcp.tile([CPC, 1], F32, tag="rstd")
                nc.vector.reciprocal(rstd, std)
                y = fcp.tile([CPC, n_ch], F32, tag="y")
                nc.vector.tensor_scalar(y, z, mv[:, 0:1], rstd[:, 0:1],
                                        op0=OP.subtract, op1=OP.mult)
                if not simple_head:
                    nc.vector.tensor_tensor(y, y, gg, OP.mult)
                    nc.vector.tensor_tensor(y, y, bb, OP.add)
                return y

            # fc1: [8,256] @ [256,512]
            z1p = psB.tile([CPC, 512], F32, tag="mix")
            for q in range(2):
                nc.tensor.matmul(z1p, (l3r[:, q, :]), (wfc1t[:, q, :]), start=(q == 0), stop=(q == 1))
            y1 = groupnorm_block(z1p, bfc1t, gn1gt, gn1bt, 512)

            # transpose y1 -> [128, 4, 8], applying relu during psum->sbuf copy
            zt1 = fcp.tile([128, 4, CPC], F32R, tag="zt1")
            for q in range(4):
                pst = psB.tile([128, CPC], F32, tag="mix")
                nc.tensor.transpose(pst, y1[:, q * 128:(q + 1) * 128], id8t)
                nc.scalar.activation(zt1[:, q, :], pst, AF.Relu)

            z2p = psB.tile([CPC, 512], F32, tag="mix")
            for q in range(4):
                nc.tensor.matmul(z2p, (zt1[:, q, :]), (wfc2t[:, q, :]), start=(q == 0), stop=(q == 3))
            y2 = groupnorm_block(z2p, bfc2t, gn2gt, gn2bt, 512)

            zt2 = fcp.tile([128, 4, CPC], F32R, tag="zt2")
            for q in range(4):
                pst = psB.tile([128, CPC], F32, tag="mix")
                nc.tensor.transpose(pst, y2[:, q * 128:(q + 1) * 128], id8t)
                nc.scalar.activation(zt2[:, q, :], pst, AF.Relu)

            ot = fcp.tile([CPC, 768], F32, tag="ot")
            for half in range(2):
                hsl = slice(half * 384, (half + 1) * 384)
                z3p = psB.tile([CPC, 384], F32, tag="mix")
                for q in range(4):
                    nc.tensor.matmul(z3p, (zt2[:, q, :]), (wfc3t[:, q, hsl]), start=(q == 0), stop=(q == 3))
                if simple_head:
                    nc.scalar.copy(out=ot[:, hsl], in_=z3p)
                else:
                    nc.vector.tensor_tensor(ot[:, hsl], z3p, bfc3t[:, hsl], OP.add)
            nc.sync.dma_start(out=out[:], in_=ot)
    nc.compile()
    return nc


# ---------------------------------------------------------------------------
# Host orchestration
# ---------------------------------------------------------------------------

_CACHE = {}


def _kernel_a(k1, relu_split=False):
    key = ("a", k1, relu_split)
    if key not in _CACHE:
        _CACHE[key] = build_kernel_a(k1, relu_split)
    return _CACHE[key]


def _kernel_b(k2, simple_head):
    key = ("b", k2, simple_head)
    if key not in _CACHE:
        _CACHE[key] = build_kernel_b(k2, simple_head)
    return _CACHE[key]


def _round_k(maxcnt, cap):
    # valid K values keep 512 % K == 0 (or slots < 512 handled by chunking)
    for k in (1, 2, 4, 8, 16, 32, 64):
        if k >= maxcnt and k <= cap:
            return k
    return cap


def kernel(xyz, params):
    xyz = np.asarray(xyz, np.float32)
    B = xyz.shape[0]
    assert B == NCORES * CPC
    pts = np.ascontiguousarray(xyz.transpose(0, 2, 1))  # [B,4096,3]
    bi = np.arange(B)[:, None, None]

    # ---- host index structure ----
    fi1 = _fps(pts, 256)
    new1 = np.take_along_axis(pts, fi1[..., None], axis=1)       # [B,256,3]
    idx1, mc1 = _ball_query(0.2, 32, pts, new1)                  # [B,256,32]
    fi2 = _fps(new1, 128)
    new2 = np.take_along_axis(new1, fi2[..., None], axis=1)      # [B,128,3]
    idx2, mc2 = _ball_query(0.4, 64, new1, new2)                 # [B,128,64]

    k1 = _round_k(mc1, 32)
    k2 = _round_k(mc2, 64)
    idx1 = np.ascontiguousarray(idx1[:, :, :k1])
    idx2 = np.ascontiguousarray(idx2[:, :, :k2])
    slots1 = 256 * k1
    slots2 = 128 * k2

    pts_g = pts[bi, idx1]                                        # [B,256,k1,3]
    rel = pts_g - new1[:, :, None, :]
    g1 = np.concatenate([rel, pts_g], axis=-1)                   # [B,256,k1,6]
    g1 = np.ascontiguousarray(g1.reshape(B, slots1, 6).transpose(0, 2, 1))

    # ---- fold weights ----
    W1, s1v, t1v = _fold_conv(params["sa1"][0])   # [64,6]
    W2, s2v, t2v = _fold_conv(params["sa1"][1])   # [64,64]
    wa = np.zeros((128, 260), np.float32)
    wa[0:6, 0:64] = W1.T
    wa[6:12, 64:128] = W1.T
    wa[0:64, 128:192] = W2.T
    wa[64:128, 192:256] = W2.T
    wa[:, 256] = np.concatenate([s1v, s1v])
    wa[:, 257] = np.concatenate([t1v, t1v])
    wa[:, 258] = np.concatenate([s2v, s2v])
    wa[:, 259] = np.concatenate([t2v, t2v])

    nca = _kernel_a(k1, False)

    # ---- kernel A ----
    g1r = g1.reshape(NCORES, NPAIR, 2, 6, slots1)
    in_maps = []
    for core in range(NCORES):
        g1p = np.empty((NPAIR, 12, slots1), np.float32)
        g1p[:, 0:6] = g1r[core, :, 0]
        g1p[:, 6:12] = g1r[core, :, 1]
        in_maps.append({"g1": g1p, "wa": wa})
    res_a = run_bass_kernel_spmd(nca, in_maps, core_ids=list(range(NCORES)))
    lo = np.stack([r["l1out"] for r in res_a.results])           # [8,4,128,256]
    l1_p = lo.reshape(NCORES, NPAIR, 2, 64, 256).transpose(0, 1, 2, 4, 3).reshape(B, 256, 64)

    # ---- host gather for SA2 groups ----
    new1_g = new1[bi, idx2]                                      # [B,128,k2,3]
    rel2 = new1_g - new2[:, :, None, :]
    pgath = l1_p[bi, idx2]                                       # [B,128,k2,64]
    g2 = np.concatenate([rel2, pgath], axis=-1)                  # [B,128,k2,67]
    g2 = np.ascontiguousarray(g2.reshape(B, slots2, 67).transpose(0, 2, 1))
    new2t = np.ascontiguousarray(new2.transpose(0, 2, 1))        # [B,3,128]

    # ---- fold SA2/SA3/FC weights into packed tensors ----
    W2a, s2av, t2av = _fold_conv(params["sa2"][0])   # [128,67]
    W2b, s2bv, t2bv = _fold_conv(params["sa2"][1])   # [128,128]
    W3a, s3av, t3av = _fold_conv(params["sa3"][0])   # [256,131]
    W3b, s3bv, t3bv = _fold_conv(params["sa3"][1])   # [256,256]
    fc1_W = np.asarray(params["fc1_W"], np.float32)  # [512,256]
    fc1_b = np.asarray(params["fc1_b"], np.float32)
    fc2_W = np.asarray(params["fc2_W"], np.float32)  # [512,512]
    fc2_b = np.asarray(params["fc2_b"], np.float32)
    fc3_W = np.asarray(params["fc3_W"], np.float32)  # [768,512]
    fc3_b = np.asarray(params["fc3_b"], np.float32)

    wbuf = np.zeros((128, WB_COLS), np.float32)
    wbuf[0:67, WB_W2A:WB_W2A + 128] = W2a.T
    wbuf[:, WB_W2B:WB_W2B + 128] = W2b.T
    wbuf[0:3, WB_W3AX:WB_W3AX + 256] = W3a[:, 0:3].T
    wbuf[:, WB_W3AP:WB_W3AP + 256] = W3a[:, 3:131].T
    wbuf[:, WB_W3B:WB_W3B + 512] = W3b.T.reshape(2, 128, 256).transpose(1, 0, 2).reshape(128, 512)
    wbuf[:, WB_WFC1:WB_WFC1 + 1024] = fc1_W.T.reshape(2, 128, 512).transpose(1, 0, 2).reshape(128, 1024)
    wbuf[:, WB_WFC2:WB_WFC2 + 2048] = fc2_W.T.reshape(4, 128, 512).transpose(1, 0, 2).reshape(128, 2048)
    wbuf[:, WB_WFC3:WB_WFC3 + 3072] = fc3_W.T.reshape(4, 128, 768).transpose(1, 0, 2).reshape(128, 3072)
    sc = WB_SC
    wbuf[:, sc + 0] = s2av
    wbuf[:, sc + 1] = t2av
    wbuf[:, sc + 2] = s2bv
    wbuf[:, sc + 3] = t2bv
    wbuf[:, sc + 4:sc + 6] = s3av.reshape(2, 128).T
    wbuf[:, sc + 6:sc + 8] = t3av.reshape(2, 128).T
    wbuf[:, sc + 8:sc + 10] = s3bv.reshape(2, 128).T
    wbuf[:, sc + 10:sc + 12] = t3bv.reshape(2, 128).T

    fcbuf = np.zeros((CPC, FCB_COLS), np.float32)
    fcbuf[:, FCB_BFC1:FCB_BFC1 + 512] = fc1_b
    fcbuf[:, FCB_GN1G:FCB_GN1G + 512] = np.asarray(params["gn1_g"], np.float32)
    fcbuf[:, FCB_GN1B:FCB_GN1B + 512] = np.asarray(params["gn1_b"], np.float32)
    fcbuf[:, FCB_BFC2:FCB_BFC2 + 512] = fc2_b
    fcbuf[:, FCB_GN2G:FCB_GN2G + 512] = np.asarray(params["gn2_g"], np.float32)
    fcbuf[:, FCB_GN2B:FCB_GN2B + 512] = np.asarray(params["gn2_b"], np.float32)
    fcbuf[:, FCB_BFC3:FCB_BFC3 + 768] = fc3_b
    fcbuf[:, FCB_ID8:FCB_ID8 + 8] = np.eye(8, dtype=np.float32)

    simple_head = bool(
        np.all(np.asarray(params["gn1_g"]) == 1) and np.all(np.asarray(params["gn1_b"]) == 0)
        and np.all(np.asarray(params["gn2_g"]) == 1) and np.all(np.asarray(params["gn2_b"]) == 0)
        and np.all(fc1_b == 0) and np.all(fc2_b == 0) and np.all(fc3_b == 0)
    )
    ncb = _kernel_b(k2, simple_head)
    in_maps_b = []
    for core in range(NCORES):
        csl = slice(core * CPC, (core + 1) * CPC)
        if k2 == 1:
            g2c = np.ascontiguousarray(g2[csl].transpose(1, 0, 2).reshape(67, CPC * 128))
            n2c = np.ascontiguousarray(new2[csl].transpose(2, 0, 1).reshape(3, CPC * 128))
        else:
            g2c = g2[csl]
            n2c = new2t[csl]
        in_maps_b.append({
            "g2": g2c,
            "new2t": n2c,
            "wb": wbuf,
            "fcb": fcbuf,
        })
    res_b = run_bass_kernel_spmd(ncb, in_maps_b, core_ids=list(range(NCORES)))
    outs = np.stack([r["out"] for r in res_b.results])           # [8,8,768]
    return outs.reshape(B, 768).reshape(B, 3, 256).astype(np.float32)


# revision 20
# speedup vs baseline: 1.0152x; 1.0152x over previous
"""Trainium2 Bass kernel for nn_AutoEncoder (PointNet++-style encoder/decoder).

Strategy (pure data parallel, B=64 clouds over 8 cores, 8 clouds/core):
  - Host (numpy): FPS sampling + ball-query + neighbor grouping — these are
    pure index functions of the input xyz and sequential/control-flow heavy.
  - Device kernel A: SA1 pointwise MLP (6->64->64) + max-pool over the K1
    group slots, two clouds batched per matmul (K=12/128, M=128), per core.
  - Host: gather SA1 features into SA2 groups (indices precomputed).
  - Device kernel B: SA2 MLP (67->128->128) + max over K2, SA3 global
    MLP (131->256->256) + max, and the FC head (256->512->512->768) with
    GroupNorm(1, C) — all per core on 8 clouds.

Key exact optimizations:
  - BatchNorm (eval) folds into relu(s*(W@x)+t); s>0 lets scale/bias/relu of
    each block's last layer commute past the max-pool.
  - Group padding slots are duplicates of a real member, and every layer is
    pointwise before a max — so groups can be truncated to the actual max
    in-radius count (K1/K2 measured on the host, kernels compiled per size).
  - Matmuls use float32r (fp32 data, fast PE mode).
"""

import numpy as np

import concourse.bass as bass
import concourse.bacc as bacc
import concourse.tile as tile
from concourse import mybir
from concourse.bass_utils import run_bass_kernel_spmd

F32 = mybir.dt.float32
F32R = mybir.dt.float32r
AF = mybir.ActivationFunctionType
AX = mybir.AxisListType
OP = mybir.AluOpType

EPS = 1e-5
INV = np.float32(1.0 / np.sqrt(1.0 + EPS))
NCORES = 8
CPC = 8  # clouds per core
NPAIR = 4  # cloud pairs per core (SA1 batches 2 clouds per matmul)

# packed const column offsets, kernel B "wb" [128, WB_COLS]
WB_W2A = 0
WB_W2B = 128
WB_W3AX = 256
WB_W3AP = 512
WB_W3B = 768        # [128, 2, 256]
WB_WFC1 = 1280      # [128, 2, 512]
WB_WFC2 = 2304      # [128, 4, 512]
WB_WFC3 = 4352      # [128, 4, 768]
WB_SC = 7424        # 12 cols: s2a,t2a,s2b,t2b,s3a0,s3a1,t3a0,t3a1,s3b0,s3b1,t3b0,t3b1
WB_COLS = 7436

# packed fc row-const offsets, kernel B "fcb" [8, FCB_COLS]
FCB_BFC1 = 0
FCB_GN1G = 512
FCB_GN1B = 1024
FCB_BFC2 = 1536
FCB_GN2G = 2048
FCB_GN2B = 2560
FCB_BFC3 = 3072
FCB_ID8 = 3840
FCB_COLS = 3848


# ---------------------------------------------------------------------------
# Host-side index math (pure functions of input xyz)
# ---------------------------------------------------------------------------


def _fps(pts, npoint):
    B, N, _ = pts.shape
    dist = np.full((B, N), 1e10, np.float32)
    far = np.zeros(B, np.int64)
    idx = np.empty((B, npoint), np.int32)
    ar = np.arange(B)
    for i in range(npoint):
        idx[:, i] = far
        c = pts[ar, far]
        d = ((pts - c[:, None, :]) ** 2).sum(-1, dtype=np.float32)
        dist = np.minimum(dist, d)
        far = dist.argmax(-1)
    return idx


def _ball_query(radius, nsample, xyz, new_xyz):
    B, N, _ = xyz.shape
    sqr = (
        (new_xyz * new_xyz).sum(-1, dtype=np.float32)[:, :, None]
        + (xyz * xyz).sum(-1, dtype=np.float32)[:, None, :]
        - np.float32(2.0) * np.einsum("bsc,bnc->bsn", new_xyz, xyz).astype(np.float32)
    )
    inr = sqr <= np.float32(radius * radius)
    cnt = inr.sum(-1)
    idx = np.where(inr, np.arange(N, dtype=np.int32), N).astype(np.int32)
    part = np.partition(idx, nsample - 1, axis=-1)[:, :, :nsample]
    part = np.sort(part, axis=-1)
    first = part[:, :, :1]
    return np.where(part == N, first, part), int(cnt.max())


def _fold_conv(layer):
    # (W,b,g,bt): layer(x) == relu(s*(W@x) + t)
    W, b, g, bt = [np.asarray(a, np.float32) for a in layer]
    s = (g * INV).astype(np.float32)
    t = (s * b + bt).astype(np.float32)
    assert (s > 0).all(), "max/scale commute needs s>0"
    return np.ascontiguousarray(W), s, t


# ---------------------------------------------------------------------------
# Bass kernel A: SA1 (6 -> 64 -> 64, max over K1) for 4 cloud-pairs
# ---------------------------------------------------------------------------


def build_kernel_a(k1, relu_split=False):
    slots = 256 * k1          # group slots per cloud
    csz = min(512, slots)     # matmul chunk width
    nch = (slots + csz - 1) // csz
    spc = csz // k1           # centers per chunk

    nc = bacc.Bacc()
    g1 = nc.dram_tensor("g1", [NPAIR, 12, slots], F32R, kind="ExternalInput")
    # packed consts: cols 0:128 w1 (rows 0:12), 128:256 w2, 256 s1, 257 t1, 258 s2, 259 t2
    wa = nc.dram_tensor("wa", [128, 260], F32R, kind="ExternalInput")
    l1out = nc.dram_tensor("l1out", [NPAIR, 128, 256], F32, kind="ExternalOutput")

    with tile.TileContext(nc) as tc:
        with (
            tc.tile_pool(name="consts", bufs=1) as consts,
            tc.tile_pool(name="gin", bufs=2) as gin,
            tc.tile_pool(name="hbuf", bufs=2) as hbuf,
            tc.tile_pool(name="obuf", bufs=2) as obuf,
            tc.tile_pool(name="ps", bufs=4, space="PSUM") as ps,
        ):
            wt = consts.tile([128, 260], F32R)
            nc.sync.dma_start(out=wt, in_=wa[:])
            w1t = wt[0:12, 0:128]
            w2t = wt[:, 128:256]
            s1t, t1t = wt[:, 256:257].bitcast(F32), wt[:, 257:258].bitcast(F32)
            s2t, t2t = wt[:, 258:259].bitcast(F32), wt[:, 259:260].bitcast(F32)

            bigw = min(1024, slots)       # psum supertile: 2 banks
            nbig = (slots + bigw - 1) // bigw
            for p in range(NPAIR):
                g = gin.tile([12, slots], F32R)
                nc.sync.dma_start(out=g, in_=g1[p])
                h1 = hbuf.tile([128, slots], F32R)
                m2 = obuf.tile([128, 256], F32, tag="m2")
                for b in range(nbig):
                    pt = ps.tile([128, bigw], F32, tag="mm")
                    for q in range(bigw // csz):
                        qs = slice(q * csz, (q + 1) * csz)
                        gs = slice(b * bigw + q * csz, b * bigw + (q + 1) * csz)
                        nc.tensor.matmul(pt[:, qs], (w1t), (g[:, gs]), start=True, stop=True)
                    bs = slice(b * bigw, (b + 1) * bigw)
                    nc.scalar.activation(h1[:, bs], pt, AF.Relu, bias=t1t, scale=s1t)
                for b in range(nbig):
                    pt2 = ps.tile([128, bigw], F32, tag="mm")
                    for q in range(bigw // csz):
                        qs = slice(q * csz, (q + 1) * csz)
                        hs = slice(b * bigw + q * csz, b * bigw + (q + 1) * csz)
                        nc.tensor.matmul(pt2[:, qs], (w2t), (h1[:, hs]), start=True, stop=True)
                    spb = bigw // k1
                    nc.vector.tensor_reduce(
                        m2[:, b * spb:(b + 1) * spb],
                        pt2.rearrange("p (s k) -> p s k", k=k1),
                        axis=AX.X,
                        op=OP.max,
                    )
                o = obuf.tile([128, 256], F32, tag="o")
                nc.scalar.activation(o, m2, AF.Relu, bias=t2t, scale=s2t)
                nc.sync.dma_start(out=l1out[p], in_=o)
    nc.compile()
    return nc


# ---------------------------------------------------------------------------
# Bass kernel B: SA2 (67 -> 128 -> 128, max over K2) + SA3 + FC head, 8 clouds
# ---------------------------------------------------------------------------


def build_kernel_b(k2, simple_head):
    slots = 128 * k2
    csz = min(512, slots)
    nch = (slots + csz - 1) // csz
    spc = csz // k2

    nc = bacc.Bacc()
    if k2 == 1:
        g2 = nc.dram_tensor("g2", [67, CPC * 128], F32R, kind="ExternalInput")
        new2t = nc.dram_tensor("new2t", [3, CPC * 128], F32R, kind="ExternalInput")
    else:
        g2 = nc.dram_tensor("g2", [CPC, 67, slots], F32R, kind="ExternalInput")
        new2t = nc.dram_tensor("new2t", [CPC, 3, 128], F32R, kind="ExternalInput")
    wb = nc.dram_tensor("wb", [128, WB_COLS], F32R, kind="ExternalInput")
    fcb = nc.dram_tensor("fcb", [CPC, FCB_COLS], F32, kind="ExternalInput")
    out = nc.dram_tensor("out", [CPC, 768], F32, kind="ExternalOutput")

    with tile.TileContext(nc) as tc:
        with (
            tc.tile_pool(name="consts", bufs=1) as consts,
            tc.tile_pool(name="gin", bufs=2) as gin,
            tc.tile_pool(name="hbuf", bufs=2) as hbuf,
            tc.tile_pool(name="small", bufs=3) as small,
            tc.tile_pool(name="fc", bufs=2) as fcp,
            tc.tile_pool(name="psA", bufs=4, space="PSUM") as psA,
            tc.tile_pool(name="psB", bufs=2, space="PSUM") as psB,
        ):
            # SA weights+scales load first (small); FC weights stream behind
            wt = consts.tile([128, 1292], F32R, tag="wb")
            nc.sync.dma_start(out=wt[:, 0:1280], in_=wb[:, 0:1280])
            nc.sync.dma_start(out=wt[:, 1280:1292], in_=wb[:, WB_SC:WB_SC + 12])
            wf1 = consts.tile([128, 1024], F32R, tag="wf1")
            nc.sync.dma_start(out=wf1, in_=wb[:, WB_WFC1:WB_WFC1 + 1024])
            wf2 = consts.tile([128, 2048], F32R, tag="wf2")
            nc.sync.dma_start(out=wf2, in_=wb[:, WB_WFC2:WB_WFC2 + 2048])
            wf3 = consts.tile([128, 3072], F32R, tag="wf3")
            nc.sync.dma_start(out=wf3, in_=wb[:, WB_WFC3:WB_WFC3 + 3072])
            fct = consts.tile([CPC, FCB_COLS], F32, tag="fcb")
            nc.sync.dma_start(out=fct, in_=fcb[:])

            w2at = wt[0:67, WB_W2A:WB_W2A + 128]
            w2bt = wt[:, WB_W2B:WB_W2B + 128]
            w3axt = wt[0:3, WB_W3AX:WB_W3AX + 256]
            w3apt = wt[:, WB_W3AP:WB_W3AP + 256]
            w3bt = wt[:, WB_W3B:WB_W3B + 512].rearrange("p (k m) -> p k m", k=2)
            wfc1t = wf1.rearrange("p (k m) -> p k m", k=2)
            wfc2t = wf2.rearrange("p (k m) -> p k m", k=4)
            wfc3t = wf3.rearrange("p (k m) -> p k m", k=4)
            sc = 1280
            s2at, t2at = wt[:, sc + 0:sc + 1].bitcast(F32), wt[:, sc + 1:sc + 2].bitcast(F32)
            s2bt, t2bt = wt[:, sc + 2:sc + 3].bitcast(F32), wt[:, sc + 3:sc + 4].bitcast(F32)
            s3at = wt[:, sc + 4:sc + 6].bitcast(F32)
            t3at = wt[:, sc + 6:sc + 8].bitcast(F32)
            s3bt = wt[:, sc + 8:sc + 10].bitcast(F32)
            t3bt = wt[:, sc + 10:sc + 12].bitcast(F32)

            bfc1t = fct[:, FCB_BFC1:FCB_BFC1 + 512]
            gn1gt = fct[:, FCB_GN1G:FCB_GN1G + 512]
            gn1bt = fct[:, FCB_GN1B:FCB_GN1B + 512]
            bfc2t = fct[:, FCB_BFC2:FCB_BFC2 + 512]
            gn2gt = fct[:, FCB_GN2G:FCB_GN2G + 512]
            gn2bt = fct[:, FCB_GN2B:FCB_GN2B + 512]
            bfc3t = fct[:, FCB_BFC3:FCB_BFC3 + 768]
            id8t = fct[0:8, FCB_ID8:FCB_ID8 + 8]

            epst = consts.tile([CPC, 1], F32, tag="eps")
            nc.vector.memset(epst, EPS)

            l3raw = consts.tile([128, 2, CPC], F32, tag="l3raw")

            if k2 == 1:
                # All 8 clouds batched along the free dim (1024 cols).
                cols = CPC * 128
                g = gin.tile([67, cols], F32R)
                nc.sync.dma_start(out=g, in_=g2[:])
                x2 = small.tile([3, cols], F32R, tag="x2")
                nc.sync.dma_start(out=x2, in_=new2t[:])
                l2p = small.tile([128, cols], F32R, tag="l2p")
                h1s = small.tile([128, cols], F32R, tag="h1s")
                for q in range(cols // 512):
                    qsl = slice(q * 512, (q + 1) * 512)
                    p1 = psA.tile([128, 512], F32, tag="mm")
                    nc.tensor.matmul(p1, (w2at), (g[:, qsl]), start=True, stop=True)
                    nc.scalar.activation(h1s[:, qsl], p1, AF.Relu, bias=t2at, scale=s2at)
                    p2 = psA.tile([128, 512], F32, tag="mm")
                    nc.tensor.matmul(p2, (w2bt), (h1s[:, qsl]), start=True, stop=True)
                    nc.scalar.activation(l2p[:, qsl], p2, AF.Relu, bias=t2bt, scale=s2bt)
                # SA3 on all clouds at once
                h3 = small.tile([128, 2, cols], F32R, tag="h3")
                for mm in range(2):
                    msl = slice(mm * 128, (mm + 1) * 128)
                    for q in range(cols // 512):
                        qsl = slice(q * 512, (q + 1) * 512)
                        p3 = psA.tile([128, 512], F32, tag="mm")
                        nc.tensor.matmul(p3, (w3axt[:, msl]), (x2[:, qsl]), start=True, stop=False)
                        nc.tensor.matmul(p3, (w3apt[:, msl]), (l2p[:, qsl]), start=False, stop=True)
                        nc.scalar.activation(
                            h3[:, mm, qsl], p3, AF.Relu,
                            bias=t3at[:, mm:mm + 1], scale=s3at[:, mm:mm + 1],
                        )
                for mm in range(2):
                    msl = slice(mm * 128, (mm + 1) * 128)
                    for q in range(cols // 512):
                        qsl = slice(q * 512, (q + 1) * 512)
                        p4 = psA.tile([128, 512], F32, tag="mm")
                        nc.tensor.matmul(p4, (w3bt[:, 0, msl]), (h3[:, 0, qsl]), start=True, stop=False)
                        nc.tensor.matmul(p4, (w3bt[:, 1, msl]), (h3[:, 1, qsl]), start=False, stop=True)
                        nc.vector.tensor_reduce(
                            l3raw[:, mm, q * 4:(q + 1) * 4],
                            p4.rearrange("p (c n) -> p c n", n=128),
                            axis=AX.X,
                            op=OP.max,
                        )
            else:
                for c in range(CPC):
                    g = gin.tile([67, slots], F32R)
                    nc.sync.dma_start(out=g, in_=g2[c])
                    l2p = small.tile([128, 128], F32R, tag="l2p")
                    h1 = hbuf.tile([128, slots], F32R)
                    m = small.tile([128, 128], F32, tag="m")
                    for fc in range(nch):
                        sl = slice(fc * csz, (fc + 1) * csz)
                        pt = psA.tile([128, csz], F32, tag="mm")
                        nc.tensor.matmul(pt, (w2at), (g[:, sl]), start=True, stop=True)
                        nc.scalar.activation(h1[:, sl], pt, AF.Relu, bias=t2at, scale=s2at)
                    for fc in range(nch):
                        sl = slice(fc * csz, (fc + 1) * csz)
                        pt2 = psA.tile([128, csz], F32, tag="mm")
                        nc.tensor.matmul(pt2, (w2bt), (h1[:, sl]), start=True, stop=True)
                        nc.vector.tensor_reduce(
                            m[:, fc * spc:(fc + 1) * spc],
                            pt2.rearrange("p (s k) -> p s k", k=k2),
                            axis=AX.X,
                            op=OP.max,
                        )
                    nc.scalar.activation(l2p, m, AF.Relu, bias=t2bt, scale=s2bt)

                    # SA3: 131 -> 256 (relu) -> 256, max over the 128 points
                    x2 = small.tile([3, 128], F32R, tag="x2")
                    nc.sync.dma_start(out=x2, in_=new2t[c])
                    h3 = small.tile([128, 2, 128], F32R, tag="h3")
                    for mm in range(2):
                        msl = slice(mm * 128, (mm + 1) * 128)
                        p3 = psB.tile([128, 128], F32, tag="mix")
                        nc.tensor.matmul(p3, (w3axt[:, msl]), (x2), start=True, stop=False)
                        nc.tensor.matmul(p3, (w3apt[:, msl]), (l2p), start=False, stop=True)
                        nc.scalar.activation(
                            h3[:, mm, :], p3, AF.Relu,
                            bias=t3at[:, mm:mm + 1], scale=s3at[:, mm:mm + 1],
                        )
                    for mm in range(2):
                        msl = slice(mm * 128, (mm + 1) * 128)
                        p4 = psB.tile([128, 128], F32, tag="mix")
                        nc.tensor.matmul(p4, (w3bt[:, 0, msl]), (h3[:, 0, :]), start=True, stop=False)
                        nc.tensor.matmul(p4, (w3bt[:, 1, msl]), (h3[:, 1, :]), start=False, stop=True)
                        nc.vector.tensor_reduce(
                            l3raw[:, mm, c:c + 1], p4, axis=AX.X, op=OP.max
                        )

            # post-max scale/bias/relu for SA3 output: [128, 2, 8]
            l3r = fcp.tile([128, 2, CPC], F32R, tag="l3r")
            for mm in range(2):
                nc.scalar.activation(
                    l3r[:, mm, :], l3raw[:, mm, :], AF.Relu,
                    bias=t3bt[:, mm:mm + 1], scale=s3bt[:, mm:mm + 1],
                )

            def groupnorm_block(z_ps, badd, gg, bb, n_ch):
                # z_ps: PSUM [8, n_ch]; returns sbuf tile [8, n_ch] = GN(z)
                # (relu is fused into the post-transpose copy)
                if simple_head:
                    z = z_ps  # bias is zero; stats straight from PSUM
                else:
                    z = fcp.tile([CPC, n_ch], F32, tag="z")
                    nc.vector.tensor_tensor(z, z_ps, badd, OP.add)
                stats = fcp.tile([CPC, 6], F32, tag="stats")
                nc.vector.bn_stats(out=stats, in_=z)
                mv = fcp.tile([CPC, 2], F32, tag="mv")
                nc.vector.bn_aggr(out=mv, in_=stats)
                std = fcp.tile([CPC, 1], F32, tag="std")
                nc.scalar.activation(std, mv[:, 1:2], AF.Sqrt, bias=epst, scale=1.0)
                rstd = fcp.tile([CPC, 1], F32, tag="rstd")
                nc.vector.reciprocal(rstd, std)
                y = fcp.tile([CPC, n_ch], F32, tag="y")
                nc.vector.tensor_scalar(y, z, mv[:, 0:1], rstd[:, 0:1],
                                        op0=OP.subtract, op1=OP.mult)
                if not simple_head:
                    nc.vector.tensor_tensor(y, y, gg, OP.mult)
                    nc.vector.tensor_tensor(y, y, bb, OP.add)
                return y

            # fc1: [8,256] @ [256,512]
            z1p = psB.tile([CPC, 512], F32, tag="mix")
            for q in range(2):
                nc.tensor.matmul(z1p, (l3r[:, q, :]), (wfc1t[:, q, :]), start=(q == 0), stop=(q == 1))
            y1 = groupnorm_block(z1p, bfc1t, gn1gt, gn1bt, 512)

            # transpose y1 -> [128, 4, 8], applying relu during psum->sbuf copy
            zt1 = fcp.tile([128, 4, CPC], F32R, tag="zt1")
            for q in range(4):
                pst = psB.tile([128, CPC], F32, tag="mix")
                nc.tensor.transpose(pst, y1[:, q * 128:(q + 1) * 128], id8t)
                nc.scalar.activation(zt1[:, q, :], pst, AF.Relu)

            z2p = psB.tile([CPC, 512], F32, tag="mix")
            for q in range(4):
                nc.tensor.matmul(z2p, (zt1[:, q, :]), (wfc2t[:, q, :]), start=(q == 0), stop=(q == 3))
            y2 = groupnorm_block(z2p, bfc2t, gn2gt, gn2bt, 512)

            zt2 = fcp.tile([128, 4, CPC], F32R, tag="zt2")
            for q in range(4):
                pst = psB.tile([128, CPC], F32, tag="mix")
                nc.tensor.transpose(pst, y2[:, q * 128:(q + 1) * 128], id8t)
                nc.scalar.activation(zt2[:, q, :], pst, AF.Relu)

            ot = fcp.tile([CPC, 768], F32, tag="ot")
            for half in range(2):
                hsl = slice(half * 384, (half + 1) * 384)
                z3p = psB.tile([CPC, 384], F32, tag="mix")
                for q in range(4):
                    nc.tensor.matmul(z3p, (zt2[:, q, :]), (wfc3t[:, q, hsl]), start=(q == 0), stop=(q == 3))
                if simple_head:
                    nc.scalar.copy(out=ot[:, hsl], in_=z3p)
                else:
                    nc.vector.tensor_tensor(ot[:, hsl], z3p, bfc3t[:, hsl], OP.add)
            nc.sync.dma_start(out=out[:], in_=ot)
    nc.compile()
    return nc


# ---------------------------------------------------------------------------
# Host orchestration
# ---------------------------------------------------------------------------

_CACHE = {}


def _kernel_a(k1, relu_split=False):
    key = ("a", k1, relu_split)
    if key not in _CACHE:
        _CACHE[key] = build_kernel_a(k1, relu_split)
    return _CACHE[key]


def _kernel_b(k2, simple_head):
    key = ("b", k2, simple_head)
    if key not in _CACHE:
        _CACHE[key] = build_kernel_b(k2, simple_head)
    return _CACHE[key]


def _round_k(maxcnt, cap):
    # valid K values keep 512 % K == 0 (or slots < 512 handled by chunking)
    for k in (1, 2, 4, 8, 16, 32, 64):
        if k >= maxcnt and k <= cap:
            return k
    return cap


def kernel(xyz, params):
    xyz = np.asarray(xyz, np.float32)
    B = xyz.shape[0]
    assert B == NCORES * CPC
    pts = np.ascontiguousarray(xyz.transpose(0, 2, 1))  # [B,4096,3]
    bi = np.arange(B)[:, None, None]

    # ---- host index structure ----
    fi1 = _fps(pts, 256)
    new1 = np.take_along_axis(pts, fi1[..., None], axis=1)       # [B,256,3]
    idx1, mc1 = _ball_query(0.2, 32, pts, new1)                  # [B,256,32]
    fi2 = _fps(new1, 128)
    new2 = np.take_along_axis(new1, fi2[..., None], axis=1)      # [B,128,3]
    idx2, mc2 = _ball_query(0.4, 64, new1, new2)                 # [B,128,64]

    k1 = _round_k(mc1, 32)
    k2 = _round_k(mc2, 64)
    idx1 = np.ascontiguousarray(idx1[:, :, :k1])
    idx2 = np.ascontiguousarray(idx2[:, :, :k2])
    slots1 = 256 * k1
    slots2 = 128 * k2

    pts_g = pts[bi, idx1]                                        # [B,256,k1,3]
    rel = pts_g - new1[:, :, None, :]
    g1 = np.concatenate([rel, pts_g], axis=-1)                   # [B,256,k1,6]
    g1 = np.ascontiguousarray(g1.reshape(B, slots1, 6).transpose(0, 2, 1))

    # ---- fold weights ----
    W1, s1v, t1v = _fold_conv(params["sa1"][0])   # [64,6]
    W2, s2v, t2v = _fold_conv(params["sa1"][1])   # [64,64]
    wa = np.zeros((128, 260), np.float32)
    wa[0:6, 0:64] = W1.T
    wa[6:12, 64:128] = W1.T
    wa[0:64, 128:192] = W2.T
    wa[64:128, 192:256] = W2.T
    wa[:, 256] = np.concatenate([s1v, s1v])
    wa[:, 257] = np.concatenate([t1v, t1v])
    wa[:, 258] = np.concatenate([s2v, s2v])
    wa[:, 259] = np.concatenate([t2v, t2v])

    nca = _kernel_a(k1, False)

    # ---- kernel A ----
    g1r = g1.reshape(NCORES, NPAIR, 2, 6, slots1)
    in_maps = []
    for core in range(NCORES):
        g1p = np.empty((NPAIR, 12, slots1), np.float32)
        g1p[:, 0:6] = g1r[core, :, 0]
        g1p[:, 6:12] = g1r[core, :, 1]
        in_maps.append({"g1": g1p, "wa": wa})
    res_a = run_bass_kernel_spmd(nca, in_maps, core_ids=list(range(NCORES)))
    lo = np.stack([r["l1out"] for r in res_a.results])           # [8,4,128,256]
    l1_p = lo.reshape(NCORES, NPAIR, 2, 64, 256).transpose(0, 1, 2, 4, 3).reshape(B, 256, 64)

    # ---- host gather for SA2 groups ----
    new1_g = new1[bi, idx2]                                      # [B,128,k2,3]
    rel2 = new1_g - new2[:, :, None, :]
    pgath = l1_p[bi, idx2]                                       # [B,128,k2,64]
    g2 = np.concatenate([rel2, pgath], axis=-1)                  # [B,128,k2,67]
    g2 = np.ascontiguousarray(g2.reshape(B, slots2, 67).transpose(0, 2, 1))
    new2t = np.ascontiguousarray(new2.transpose(0, 2, 1))        # [B,3,128]

    # ---- fold SA2/SA3/FC weights into packed tensors ----
    W2a, s2av, t2av = _fold_conv(params["sa2"][0])   # [128,67]
    W2b, s2bv, t2bv = _fold_conv(params["sa2"][1])   # [128,128]
    W3a, s3av, t3av = _fold_conv(params["sa3"][0])   # [256,131]
    W3b, s3bv, t3bv = _fold_conv(params["sa3"][1])   # [256,256]
    fc1_W = np.asarray(params["fc1_W"], np.float32)  # [512,256]
    fc1_b = np.asarray(params["fc1_b"], np.float32)
    fc2_W = np.asarray(params["fc2_W"], np.float32)  # [512,512]
    fc2_b = np.asarray(params["fc2_b"], np.float32)
    fc3_W = np.asarray(params["fc3_W"], np.float32)  # [768,512]
    fc3_b = np.asarray(params["fc3_b"], np.float32)

    wbuf = np.zeros((128, WB_COLS), np.float32)
    wbuf[0:67, WB_W2A:WB_W2A + 128] = W2a.T
    wbuf[:, WB_W2B:WB_W2B + 128] = W2b.T
    wbuf[0:3, WB_W3AX:WB_W3AX + 256] = W3a[:, 0:3].T
    wbuf[:, WB_W3AP:WB_W3AP + 256] = W3a[:, 3:131].T
    wbuf[:, WB_W3B:WB_W3B + 512] = W3b.T.reshape(2, 128, 256).transpose(1, 0, 2).reshape(128, 512)
    wbuf[:, WB_WFC1:WB_WFC1 + 1024] = fc1_W.T.reshape(2, 128, 512).transpose(1, 0, 2).reshape(128, 1024)
    wbuf[:, WB_WFC2:WB_WFC2 + 2048] = fc2_W.T.reshape(4, 128, 512).transpose(1, 0, 2).reshape(128, 2048)
    wbuf[:, WB_WFC3:WB_WFC3 + 3072] = fc3_W.T.reshape(4, 128, 768).transpose(1, 0, 2).reshape(128, 3072)
    sc = WB_SC
    wbuf[:, sc + 0] = s2av
    wbuf[:, sc + 1] = t2av
    wbuf[:, sc + 2] = s2bv
    wbuf[:, sc + 3] = t2bv
    wbuf[:, sc + 4:sc + 6] = s3av.reshape(2, 128).T
    wbuf[:, sc + 6:sc + 8] = t3av.reshape(2, 128).T
    wbuf[:, sc + 8:sc + 10] = s3bv.reshape(2, 128).T
    wbuf[:, sc + 10:sc + 12] = t3bv.reshape(2, 128).T

    fcbuf = np.zeros((CPC, FCB_COLS), np.float32)
    fcbuf[:, FCB_BFC1:FCB_BFC1 + 512] = fc1_b
    fcbuf[:, FCB_GN1G:FCB_GN1G + 512] = np.asarray(params["gn1_g"], np.float32)
    fcbuf[:, FCB_GN1B:FCB_GN1B + 512] = np.asarray(params["gn1_b"], np.float32)
    fcbuf[:, FCB_BFC2:FCB_BFC2 + 512] = fc2_b
    fcbuf[:, FCB_GN2G:FCB_GN2G + 512] = np.asarray(params["gn2_g"], np.float32)
    fcbuf[:, FCB_GN2B:FCB_GN2B + 512] = np.asarray(params["gn2_b"], np.float32)
    fcbuf[:, FCB_BFC3:FCB_BFC3 + 768] = fc3_b
    fcbuf[:, FCB_ID8:FCB_ID8 + 8] = np.eye(8, dtype=np.float32)

    simple_head = bool(
        np.all(np.asarray(params["gn1_g"]) == 1) and np.all(np.asarray(params["gn1_b"]) == 0)
        and np.all(np.asarray(params["gn2_g"]) == 1) and np.all(np.asarray(params["gn2_b"]) == 0)
        and np.all(fc1_b == 0) and np.all(fc2_b == 0) and np.all(fc3_b == 0)
    )
    ncb = _kernel_b(k2, simple_head)
    in_maps_b = []
    for core in range(NCORES):
        csl = slice(core * CPC, (core + 1) * CPC)
        if k2 == 1:
            g2c = np.ascontiguousarray(g2[csl].transpose(1, 0, 2).reshape(67, CPC * 128))
            n2c = np.ascontiguousarray(new2[csl].transpose(2, 0, 1).reshape(3, CPC * 128))
        else:
            g2c = g2[csl]
            n2c = new2t[csl]
        in_maps_b.append({
            "g2": g2c,
            "new2t": n2c,
            "wb": wbuf,
            "fcb": fcbuf,
        })
    res_b = run_bass_kernel_spmd(ncb, in_maps_b, core_ids=list(range(NCORES)))
    outs = np.stack([r["out"] for r in res_b.results])           # [8,8,768]
    return outs.reshape(B, 768).reshape(B, 3, 256).astype(np.float32)


# revision 21
# speedup vs baseline: 1.0297x; 1.0143x over previous
"""Trainium2 Bass kernel for nn_AutoEncoder (PointNet++-style encoder/decoder).

Strategy (pure data parallel, B=64 clouds over 8 cores, 8 clouds/core):
  - Host (numpy): FPS sampling + ball-query + neighbor grouping — these are
    pure index functions of the input xyz and sequential/control-flow heavy.
  - Device kernel A: SA1 pointwise MLP (6->64->64) + max-pool over the K1
    group slots, two clouds batched per matmul (K=12/128, M=128), per core.
  - Host: gather SA1 features into SA2 groups (indices precomputed).
  - Device kernel B: SA2 MLP (67->128->128) + max over K2, SA3 global
    MLP (131->256->256) + max, and the FC head (256->512->512->768) with
    GroupNorm(1, C) — all per core on 8 clouds.

Key exact optimizations:
  - BatchNorm (eval) folds into relu(s*(W@x)+t); s>0 lets scale/bias/relu of
    each block's last layer commute past the max-pool.
  - Group padding slots are duplicates of a real member, and every layer is
    pointwise before a max — so groups can be truncated to the actual max
    in-radius count (K1/K2 measured on the host, kernels compiled per size).
  - Matmuls use float32r (fp32 data, fast PE mode).
"""

import numpy as np

import concourse.bass as bass
import concourse.bacc as bacc
import concourse.tile as tile
from concourse import mybir
from concourse.bass_utils import run_bass_kernel_spmd

F32 = mybir.dt.float32
F32R = mybir.dt.float32r
AF = mybir.ActivationFunctionType
AX = mybir.AxisListType
OP = mybir.AluOpType

EPS = 1e-5
INV = np.float32(1.0 / np.sqrt(1.0 + EPS))
NCORES = 8
CPC = 8  # clouds per core
NPAIR = 4  # cloud pairs per core (SA1 batches 2 clouds per matmul)

# packed const column offsets, kernel B "wb" [128, WB_COLS]
WB_W2A = 0
WB_W2B = 128
WB_W3AX = 256
WB_W3AP = 512
WB_W3B = 768        # [128, 2, 256]
WB_WFC1 = 1280      # [128, 2, 512]
WB_WFC2 = 2304      # [128, 4, 512]
WB_WFC3 = 4352      # [128, 4, 768]
WB_SC = 7424        # 12 cols: s2a,t2a,s2b,t2b,s3a0,s3a1,t3a0,t3a1,s3b0,s3b1,t3b0,t3b1
WB_COLS = 7436

# packed fc row-const offsets, kernel B "fcb" [8, FCB_COLS]
FCB_BFC1 = 0
FCB_GN1G = 512
FCB_GN1B = 1024
FCB_BFC2 = 1536
FCB_GN2G = 2048
FCB_GN2B = 2560
FCB_BFC3 = 3072
FCB_ID8 = 3840
FCB_COLS = 3848


# ---------------------------------------------------------------------------
# Host-side index math (pure functions of input xyz)
# ---------------------------------------------------------------------------


def _fps(pts, npoint):
    B, N, _ = pts.shape
    dist = np.full((B, N), 1e10, np.float32)
    far = np.zeros(B, np.int64)
    idx = np.empty((B, npoint), np.int32)
    ar = np.arange(B)
    for i in range(npoint):
        idx[:, i] = far
        c = pts[ar, far]
        d = ((pts - c[:, None, :]) ** 2).sum(-1, dtype=np.float32)
        dist = np.minimum(dist, d)
        far = dist.argmax(-1)
    return idx


def _ball_query(radius, nsample, xyz, new_xyz):
    B, N, _ = xyz.shape
    sqr = (
        (new_xyz * new_xyz).sum(-1, dtype=np.float32)[:, :, None]
        + (xyz * xyz).sum(-1, dtype=np.float32)[:, None, :]
        - np.float32(2.0) * np.einsum("bsc,bnc->bsn", new_xyz, xyz).astype(np.float32)
    )
    inr = sqr <= np.float32(radius * radius)
    cnt = inr.sum(-1)
    idx = np.where(inr, np.arange(N, dtype=np.int32), N).astype(np.int32)
    part = np.partition(idx, nsample - 1, axis=-1)[:, :, :nsample]
    part = np.sort(part, axis=-1)
    first = part[:, :, :1]
    return np.where(part == N, first, part), int(cnt.max())


def _fold_conv(layer):
    # (W,b,g,bt): layer(x) == relu(s*(W@x) + t)
    W, b, g, bt = [np.asarray(a, np.float32) for a in layer]
    s = (g * INV).astype(np.float32)
    t = (s * b + bt).astype(np.float32)
    assert (s > 0).all(), "max/scale commute needs s>0"
    return np.ascontiguousarray(W), s, t


# ---------------------------------------------------------------------------
# Bass kernel A: SA1 (6 -> 64 -> 64, max over K1) for 4 cloud-pairs
# ---------------------------------------------------------------------------


def build_kernel_a(k1, relu_split=False):
    slots = 256 * k1          # group slots per cloud
    csz = min(512, slots)     # matmul chunk width
    nch = (slots + csz - 1) // csz
    spc = csz // k1           # centers per chunk

    nc = bacc.Bacc()
    g1 = nc.dram_tensor("g1", [NPAIR, 12, slots], F32R, kind="ExternalInput")
    # packed consts: cols 0:128 w1 (rows 0:12), 128:256 w2, 256 s1, 257 t1, 258 s2, 259 t2
    wa = nc.dram_tensor("wa", [128, 260], F32R, kind="ExternalInput")
    l1out = nc.dram_tensor("l1out", [NPAIR, 128, 256], F32, kind="ExternalOutput")

    with tile.TileContext(nc) as tc:
        with (
            tc.tile_pool(name="consts", bufs=1) as consts,
            tc.tile_pool(name="gin", bufs=3) as gin,
            tc.tile_pool(name="hbuf", bufs=3) as hbuf,
            tc.tile_pool(name="obuf", bufs=3) as obuf,
            tc.tile_pool(name="ps", bufs=4, space="PSUM") as ps,
        ):
            wt = consts.tile([128, 260], F32R)
            nc.sync.dma_start(out=wt, in_=wa[:])
            w1t = wt[0:12, 0:128]
            w2t = wt[:, 128:256]
            s1t, t1t = wt[:, 256:257].bitcast(F32), wt[:, 257:258].bitcast(F32)
            s2t, t2t = wt[:, 258:259].bitcast(F32), wt[:, 259:260].bitcast(F32)

            bigw = min(1024, slots)       # psum supertile: 2 banks
            nbig = (slots + bigw - 1) // bigw
            for p in range(NPAIR):
                g = gin.tile([12, slots], F32R)
                nc.sync.dma_start(out=g, in_=g1[p])
                h1 = hbuf.tile([128, slots], F32R)
                m2 = obuf.tile([128, 256], F32, tag="m2")
                for b in range(nbig):
                    pt = ps.tile([128, bigw], F32, tag="mm")
                    for q in range(bigw // csz):
                        qs = slice(q * csz, (q + 1) * csz)
                        gs = slice(b * bigw + q * csz, b * bigw + (q + 1) * csz)
                        nc.tensor.matmul(pt[:, qs], (w1t), (g[:, gs]), start=True, stop=True)
                    bs = slice(b * bigw, (b + 1) * bigw)
                    nc.scalar.activation(h1[:, bs], pt, AF.Relu, bias=t1t, scale=s1t)
                for b in range(nbig):
                    pt2 = ps.tile([128, bigw], F32, tag="mm")
                    for q in range(bigw // csz):
                        qs = slice(q * csz, (q + 1) * csz)
                        hs = slice(b * bigw + q * csz, b * bigw + (q + 1) * csz)
                        nc.tensor.matmul(pt2[:, qs], (w2t), (h1[:, hs]), start=True, stop=True)
                    spb = bigw // k1
                    nc.vector.tensor_reduce(
                        m2[:, b * spb:(b + 1) * spb],
                        pt2.rearrange("p (s k) -> p s k", k=k1),
                        axis=AX.X,
                        op=OP.max,
                    )
                o = obuf.tile([128, 256], F32, tag="o")
                nc.scalar.activation(o, m2, AF.Relu, bias=t2t, scale=s2t)
                nc.sync.dma_start(out=l1out[p], in_=o)
    nc.compile()
    return nc


# ---------------------------------------------------------------------------
# Bass kernel B: SA2 (67 -> 128 -> 128, max over K2) + SA3 + FC head, 8 clouds
# ---------------------------------------------------------------------------


def build_kernel_b(k2, simple_head):
    slots = 128 * k2
    csz = min(512, slots)
    nch = (slots + csz - 1) // csz
    spc = csz // k2

    nc = bacc.Bacc()
    if k2 == 1:
        g2 = nc.dram_tensor("g2", [67, CPC * 128], F32R, kind="ExternalInput")
        new2t = nc.dram_tensor("new2t", [3, CPC * 128], F32R, kind="ExternalInput")
    else:
        g2 = nc.dram_tensor("g2", [CPC, 67, slots], F32R, kind="ExternalInput")
        new2t = nc.dram_tensor("new2t", [CPC, 3, 128], F32R, kind="ExternalInput")
    wb = nc.dram_tensor("wb", [128, WB_COLS], F32R, kind="ExternalInput")
    fcb = nc.dram_tensor("fcb", [CPC, FCB_COLS], F32, kind="ExternalInput")
    out = nc.dram_tensor("out", [CPC, 768], F32, kind="ExternalOutput")

    with tile.TileContext(nc) as tc:
        with (
            tc.tile_pool(name="consts", bufs=1) as consts,
            tc.tile_pool(name="gin", bufs=2) as gin,
            tc.tile_pool(name="hbuf", bufs=2) as hbuf,
            tc.tile_pool(name="small", bufs=3) as small,
            tc.tile_pool(name="fc", bufs=2) as fcp,
            tc.tile_pool(name="psA", bufs=4, space="PSUM") as psA,
            tc.tile_pool(name="psB", bufs=3, space="PSUM") as psB,
        ):
            # SA weights+scales load first (small); FC weights stream behind
            wt = consts.tile([128, 1292], F32R, tag="wb")
            nc.sync.dma_start(out=wt[:, 0:1280], in_=wb[:, 0:1280])
            nc.sync.dma_start(out=wt[:, 1280:1292], in_=wb[:, WB_SC:WB_SC + 12])
            wf1 = consts.tile([128, 1024], F32R, tag="wf1")
            nc.sync.dma_start(out=wf1, in_=wb[:, WB_WFC1:WB_WFC1 + 1024])
            wf2 = consts.tile([128, 2048], F32R, tag="wf2")
            nc.sync.dma_start(out=wf2, in_=wb[:, WB_WFC2:WB_WFC2 + 2048])
            wf3 = consts.tile([128, 3072], F32R, tag="wf3")
            nc.sync.dma_start(out=wf3, in_=wb[:, WB_WFC3:WB_WFC3 + 3072])
            fct = consts.tile([CPC, FCB_COLS], F32, tag="fcb")
            nc.sync.dma_start(out=fct, in_=fcb[:])

            w2at = wt[0:67, WB_W2A:WB_W2A + 128]
            w2bt = wt[:, WB_W2B:WB_W2B + 128]
            w3axt = wt[0:3, WB_W3AX:WB_W3AX + 256]
            w3apt = wt[:, WB_W3AP:WB_W3AP + 256]
            w3bt = wt[:, WB_W3B:WB_W3B + 512].rearrange("p (k m) -> p k m", k=2)
            wfc1t = wf1.rearrange("p (k m) -> p k m", k=2)
            wfc2t = wf2.rearrange("p (k m) -> p k m", k=4)
            wfc3t = wf3.rearrange("p (k m) -> p k m", k=4)
            sc = 1280
            s2at, t2at = wt[:, sc + 0:sc + 1].bitcast(F32), wt[:, sc + 1:sc + 2].bitcast(F32)
            s2bt, t2bt = wt[:, sc + 2:sc + 3].bitcast(F32), wt[:, sc + 3:sc + 4].bitcast(F32)
            s3at = wt[:, sc + 4:sc + 6].bitcast(F32)
            t3at = wt[:, sc + 6:sc + 8].bitcast(F32)
            s3bt = wt[:, sc + 8:sc + 10].bitcast(F32)
            t3bt = wt[:, sc + 10:sc + 12].bitcast(F32)

            bfc1t = fct[:, FCB_BFC1:FCB_BFC1 + 512]
            gn1gt = fct[:, FCB_GN1G:FCB_GN1G + 512]
            gn1bt = fct[:, FCB_GN1B:FCB_GN1B + 512]
            bfc2t = fct[:, FCB_BFC2:FCB_BFC2 + 512]
            gn2gt = fct[:, FCB_GN2G:FCB_GN2G + 512]
            gn2bt = fct[:, FCB_GN2B:FCB_GN2B + 512]
            bfc3t = fct[:, FCB_BFC3:FCB_BFC3 + 768]
            id8t = fct[0:8, FCB_ID8:FCB_ID8 + 8]

            epst = consts.tile([CPC, 1], F32, tag="eps")
            nc.vector.memset(epst, EPS)

            l3raw = consts.tile([128, 2, CPC], F32, tag="l3raw")

            if k2 == 1:
                # All 8 clouds batched along the free dim (1024 cols).
                cols = CPC * 128
                g = gin.tile([67, cols], F32R)
                nc.sync.dma_start(out=g, in_=g2[:])
                x2 = small.tile([3, cols], F32R, tag="x2")
                nc.sync.dma_start(out=x2, in_=new2t[:])
                l2p = small.tile([128, cols], F32R, tag="l2p")
                h1s = small.tile([128, cols], F32R, tag="h1s")
                for q in range(cols // 512):
                    qsl = slice(q * 512, (q + 1) * 512)
                    p1 = psA.tile([128, 512], F32, tag="mm")
                    nc.tensor.matmul(p1, (w2at), (g[:, qsl]), start=True, stop=True)
                    nc.scalar.activation(h1s[:, qsl], p1, AF.Relu, bias=t2at, scale=s2at)
                    p2 = psA.tile([128, 512], F32, tag="mm")
                    nc.tensor.matmul(p2, (w2bt), (h1s[:, qsl]), start=True, stop=True)
                    nc.scalar.activation(l2p[:, qsl], p2, AF.Relu, bias=t2bt, scale=s2bt)
                # SA3 on all clouds at once
                h3 = small.tile([128, 2, cols], F32R, tag="h3")
                for mm in range(2):
                    msl = slice(mm * 128, (mm + 1) * 128)
                    for q in range(cols // 512):
                        qsl = slice(q * 512, (q + 1) * 512)
                        p3 = psA.tile([128, 512], F32, tag="mm")
                        nc.tensor.matmul(p3, (w3axt[:, msl]), (x2[:, qsl]), start=True, stop=False)
                        nc.tensor.matmul(p3, (w3apt[:, msl]), (l2p[:, qsl]), start=False, stop=True)
                        nc.scalar.activation(
                            h3[:, mm, qsl], p3, AF.Relu,
                            bias=t3at[:, mm:mm + 1], scale=s3at[:, mm:mm + 1],
                        )
                for mm in range(2):
                    msl = slice(mm * 128, (mm + 1) * 128)
                    for q in range(cols // 512):
                        qsl = slice(q * 512, (q + 1) * 512)
                        p4 = psA.tile([128, 512], F32, tag="mm")
                        nc.tensor.matmul(p4, (w3bt[:, 0, msl]), (h3[:, 0, qsl]), start=True, stop=False)
                        nc.tensor.matmul(p4, (w3bt[:, 1, msl]), (h3[:, 1, qsl]), start=False, stop=True)
                        nc.vector.tensor_reduce(
                            l3raw[:, mm, q * 4:(q + 1) * 4],
                            p4.rearrange("p (c n) -> p c n", n=128),
                            axis=AX.X,
                            op=OP.max,
                        )
            else:
                for c in range(CPC):
                    g = gin.tile([67, slots], F32R)
                    nc.sync.dma_start(out=g, in_=g2[c])
                    l2p = small.tile([128, 128], F32R, tag="l2p")
                    h1 = hbuf.tile([128, slots], F32R)
                    m = small.tile([128, 128], F32, tag="m")
                    for fc in range(nch):
                        sl = slice(fc * csz, (fc + 1) * csz)
                        pt = psA.tile([128, csz], F32, tag="mm")
                        nc.tensor.matmul(pt, (w2at), (g[:, sl]), start=True, stop=True)
                        nc.scalar.activation(h1[:, sl], pt, AF.Relu, bias=t2at, scale=s2at)
                    for fc in range(nch):
                        sl = slice(fc * csz, (fc + 1) * csz)
                        pt2 = psA.tile([128, csz], F32, tag="mm")
                        nc.tensor.matmul(pt2, (w2bt), (h1[:, sl]), start=True, stop=True)
                        nc.vector.tensor_reduce(
                            m[:, fc * spc:(fc + 1) * spc],
                            pt2.rearrange("p (s k) -> p s k", k=k2),
                            axis=AX.X,
                            op=OP.max,
                        )
                    nc.scalar.activation(l2p, m, AF.Relu, bias=t2bt, scale=s2bt)

                    # SA3: 131 -> 256 (relu) -> 256, max over the 128 points
                    x2 = small.tile([3, 128], F32R, tag="x2")
                    nc.sync.dma_start(out=x2, in_=new2t[c])
                    h3 = small.tile([128, 2, 128], F32R, tag="h3")
                    for mm in range(2):
                        msl = slice(mm * 128, (mm + 1) * 128)
                        p3 = psB.tile([128, 128], F32, tag="mix")
                        nc.tensor.matmul(p3, (w3axt[:, msl]), (x2), start=True, stop=False)
                        nc.tensor.matmul(p3, (w3apt[:, msl]), (l2p), start=False, stop=True)
                        nc.scalar.activation(
                            h3[:, mm, :], p3, AF.Relu,
                            bias=t3at[:, mm:mm + 1], scale=s3at[:, mm:mm + 1],
                        )
                    for mm in range(2):
                        msl = slice(mm * 128, (mm + 1) * 128)
                        p4 = psB.tile([128, 128], F32, tag="mix")
                        nc.tensor.matmul(p4, (w3bt[:, 0, msl]), (h3[:, 0, :]), start=True, stop=False)
                        nc.tensor.matmul(p4, (w3bt[:, 1, msl]), (h3[:, 1, :]), start=False, stop=True)
                        nc.vector.tensor_reduce(
                            l3raw[:, mm, c:c + 1], p4, axis=AX.X, op=OP.max
                        )

            # post-max scale/bias/relu for SA3 output: [128, 2, 8]
            l3r = fcp.tile([128, 2, CPC], F32R, tag="l3r")
            for mm in range(2):
                nc.scalar.activation(
                    l3r[:, mm, :], l3raw[:, mm, :], AF.Relu,
                    bias=t3bt[:, mm:mm + 1], scale=s3bt[:, mm:mm + 1],
                )

            def groupnorm_block(z_ps, badd, gg, bb, n_ch):
                # z_ps: PSUM [8, n_ch]; returns sbuf tile [8, n_ch] = GN(z)
                # (relu is fused into the post-transpose copy)
                if simple_head:
                    z = z_ps  # bias is zero; stats straight from PSUM
                else:
                    z = fcp.tile([CPC, n_ch], F32, tag="z")
                    nc.vector.tensor_tensor(z, z_ps, badd, OP.add)
                stats = fcp.tile([CPC, 6], F32, tag="stats")
                nc.vector.bn_stats(out=stats, in_=z)
                mv = fcp.tile([CPC, 2], F32, tag="mv")
                nc.vector.bn_aggr(out=mv, in_=stats)
                std = fcp.tile([CPC, 1], F32, tag="std")
                nc.scalar.activation(std, mv[:, 1:2], AF.Sqrt, bias=epst, scale=1.0)
                rstd = fcp.tile([CPC, 1], F32, tag="rstd")
                nc.vector.reciprocal(rstd, std)
                y = fcp.tile([CPC, n_ch], F32, tag="y")
                nc.vector.tensor_scalar(y, z, mv[:, 0:1], rstd[:, 0:1],
                                        op0=OP.subtract, op1=OP.mult)
                if not simple_head:
                    nc.vector.tensor_tensor(y, y, gg, OP.mult)
                    nc.vector.tensor_tensor(y, y, bb, OP.add)
                return y

            # fc1: [8,256] @ [256,512]
            z1p = psB.tile([CPC, 512], F32, tag="mix")
            for q in range(2):
                nc.tensor.matmul(z1p, (l3r[:, q, :]), (wfc1t[:, q, :]), start=(q == 0), stop=(q == 1))
            y1 = groupnorm_block(z1p, bfc1t, gn1gt, gn1bt, 512)

            # transpose y1 -> [128, 4, 8], applying relu during psum->sbuf copy
            zt1 = fcp.tile([128, 4, CPC], F32R, tag="zt1")
            for q in range(4):
                pst = psB.tile([128, CPC], F32, tag="mix")
                nc.tensor.transpose(pst, y1[:, q * 128:(q + 1) * 128], id8t)
                nc.scalar.activation(zt1[:, q, :], pst, AF.Relu)

            z2p = psB.tile([CPC, 512], F32, tag="mix")
            for q in range(4):
                nc.tensor.matmul(z2p, (zt1[:, q, :]), (wfc2t[:, q, :]), start=(q == 0), stop=(q == 3))
            y2 = groupnorm_block(z2p, bfc2t, gn2gt, gn2bt, 512)

            zt2 = fcp.tile([128, 4, CPC], F32R, tag="zt2")
            for q in range(4):
                pst = psB.tile([128, CPC], F32, tag="mix")
                nc.tensor.transpose(pst, y2[:, q * 128:(q + 1) * 128], id8t)
                nc.scalar.activation(zt2[:, q, :], pst, AF.Relu)

            ot = fcp.tile([CPC, 768], F32, tag="ot")
            for half in range(2):
                hsl = slice(half * 384, (half + 1) * 384)
                z3p = psB.tile([CPC, 384], F32, tag="mix")
                for q in range(4):
                    nc.tensor.matmul(z3p, (zt2[:, q, :]), (wfc3t[:, q, hsl]), start=(q == 0), stop=(q == 3))
                if simple_head:
                    nc.scalar.copy(out=ot[:, hsl], in_=z3p)
                else:
                    nc.vector.tensor_tensor(ot[:, hsl], z3p, bfc3t[:, hsl], OP.add)
            nc.sync.dma_start(out=out[:], in_=ot)
    nc.compile()
    return nc


# ---------------------------------------------------------------------------
# Host orchestration
# ---------------------------------------------------------------------------

_CACHE = {}


def _kernel_a(k1, relu_split=False):
    key = ("a", k1, relu_split)
    if key not in _CACHE:
        _CACHE[key] = build_kernel_a(k1, relu_split)
    return _CACHE[key]


def _kernel_b(k2, simple_head):
    key = ("b", k2, simple_head)
    if key not in _CACHE:
        _CACHE[key] = build_kernel_b(k2, simple_head)
    return _CACHE[key]


def _round_k(maxcnt, cap):
    # valid K values keep 512 % K == 0 (or slots < 512 handled by chunking)
    for k in (1, 2, 4, 8, 16, 32, 64):
        if k >= maxcnt and k <= cap:
            return k
    return cap


def kernel(xyz, params):
    xyz = np.asarray(xyz, np.float32)
    B = xyz.shape[0]
    assert B == NCORES * CPC
    pts = np.ascontiguousarray(xyz.transpose(0, 2, 1))  # [B,4096,3]
    bi = np.arange(B)[:, None, None]

    # ---- host index structure ----
    fi1 = _fps(pts, 256)
    new1 = np.take_along_axis(pts, fi1[..., None], axis=1)       # [B,256,3]
    idx1, mc1 = _ball_query(0.2, 32, pts, new1)                  # [B,256,32]
    fi2 = _fps(new1, 128)
    new2 = np.take_along_axis(new1, fi2[..., None], axis=1)      # [B,128,3]
    idx2, mc2 = _ball_query(0.4, 64, new1, new2)                 # [B,128,64]

    k1 = _round_k(mc1, 32)
    k2 = _round_k(mc2, 64)
    idx1 = np.ascontiguousarray(idx1[:, :, :k1])
    idx2 = np.ascontiguousarray(idx2[:, :, :k2])
    slots1 = 256 * k1
    slots2 = 128 * k2

    pts_g = pts[bi, idx1]                                        # [B,256,k1,3]
    rel = pts_g - new1[:, :, None, :]
    g1 = np.concatenate([rel, pts_g], axis=-1)                   # [B,256,k1,6]
    g1 = np.ascontiguousarray(g1.reshape(B, slots1, 6).transpose(0, 2, 1))

    # ---- fold weights ----
    W1, s1v, t1v = _fold_conv(params["sa1"][0])   # [64,6]
    W2, s2v, t2v = _fold_conv(params["sa1"][1])   # [64,64]
    wa = np.zeros((128, 260), np.float32)
    wa[0:6, 0:64] = W1.T
    wa[6:12, 64:128] = W1.T
    wa[0:64, 128:192] = W2.T
    wa[64:128, 192:256] = W2.T
    wa[:, 256] = np.concatenate([s1v, s1v])
    wa[:, 257] = np.concatenate([t1v, t1v])
    wa[:, 258] = np.concatenate([s2v, s2v])
    wa[:, 259] = np.concatenate([t2v, t2v])

    nca = _kernel_a(k1, False)

    # ---- kernel A ----
    g1r = g1.reshape(NCORES, NPAIR, 2, 6, slots1)
    in_maps = []
    for core in range(NCORES):
        g1p = np.empty((NPAIR, 12, slots1), np.float32)
        g1p[:, 0:6] = g1r[core, :, 0]
        g1p[:, 6:12] = g1r[core, :, 1]
        in_maps.append({"g1": g1p, "wa": wa})
    res_a = run_bass_kernel_spmd(nca, in_maps, core_ids=list(range(NCORES)))
    lo = np.stack([r["l1out"] for r in res_a.results])           # [8,4,128,256]
    l1_p = lo.reshape(NCORES, NPAIR, 2, 64, 256).transpose(0, 1, 2, 4, 3).reshape(B, 256, 64)

    # ---- host gather for SA2 groups ----
    new1_g = new1[bi, idx2]                                      # [B,128,k2,3]
    rel2 = new1_g - new2[:, :, None, :]
    pgath = l1_p[bi, idx2]                                       # [B,128,k2,64]
    g2 = np.concatenate([rel2, pgath], axis=-1)                  # [B,128,k2,67]
    g2 = np.ascontiguousarray(g2.reshape(B, slots2, 67).transpose(0, 2, 1))
    new2t = np.ascontiguousarray(new2.transpose(0, 2, 1))        # [B,3,128]

    # ---- fold SA2/SA3/FC weights into packed tensors ----
    W2a, s2av, t2av = _fold_conv(params["sa2"][0])   # [128,67]
    W2b, s2bv, t2bv = _fold_conv(params["sa2"][1])   # [128,128]
    W3a, s3av, t3av = _fold_conv(params["sa3"][0])   # [256,131]
    W3b, s3bv, t3bv = _fold_conv(params["sa3"][1])   # [256,256]
    fc1_W = np.asarray(params["fc1_W"], np.float32)  # [512,256]
    fc1_b = np.asarray(params["fc1_b"], np.float32)
    fc2_W = np.asarray(params["fc2_W"], np.float32)  # [512,512]
    fc2_b = np.asarray(params["fc2_b"], np.float32)
    fc3_W = np.asarray(params["fc3_W"], np.float32)  # [768,512]
    fc3_b = np.asarray(params["fc3_b"], np.float32)

    wbuf = np.zeros((128, WB_COLS), np.float32)
    wbuf[0:67, WB_W2A:WB_W2A + 128] = W2a.T
    wbuf[:, WB_W2B:WB_W2B + 128] = W2b.T
    wbuf[0:3, WB_W3AX:WB_W3AX + 256] = W3a[:, 0:3].T
    wbuf[:, WB_W3AP:WB_W3AP + 256] = W3a[:, 3:131].T
    wbuf[:, WB_W3B:WB_W3B + 512] = W3b.T.reshape(2, 128, 256).transpose(1, 0, 2).reshape(128, 512)
    wbuf[:, WB_WFC1:WB_WFC1 + 1024] = fc1_W.T.reshape(2, 128, 512).transpose(1, 0, 2).reshape(128, 1024)
    wbuf[:, WB_WFC2:WB_WFC2 + 2048] = fc2_W.T.reshape(4, 128, 512).transpose(1, 0, 2).reshape(128, 2048)
    wbuf[:, WB_WFC3:WB_WFC3 + 3072] = fc3_W.T.reshape(4, 128, 768).transpose(1, 0, 2).reshape(128, 3072)
    sc = WB_SC
    wbuf[:, sc + 0] = s2av
    wbuf[:, sc + 1] = t2av
    wbuf[:, sc + 2] = s2bv
    wbuf[:, sc + 3] = t2bv
    wbuf[:, sc + 4:sc + 6] = s3av.reshape(2, 128).T
    wbuf[:, sc + 6:sc + 8] = t3av.reshape(2, 128).T
    wbuf[:, sc + 8:sc + 10] = s3bv.reshape(2, 128).T
    wbuf[:, sc + 10:sc + 12] = t3bv.reshape(2, 128).T

    fcbuf = np.zeros((CPC, FCB_COLS), np.float32)
    fcbuf[:, FCB_BFC1:FCB_BFC1 + 512] = fc1_b
    fcbuf[:, FCB_GN1G:FCB_GN1G + 512] = np.asarray(params["gn1_g"], np.float32)
    fcbuf[:, FCB_GN1B:FCB_GN1B + 512] = np.asarray(params["gn1_b"], np.float32)
    fcbuf[:, FCB_BFC2:FCB_BFC2 + 512] = fc2_b
    fcbuf[:, FCB_GN2G:FCB_GN2G + 512] = np.asarray(params["gn2_g"], np.float32)
    fcbuf[:, FCB_GN2B:FCB_GN2B + 512] = np.asarray(params["gn2_b"], np.float32)
    fcbuf[:, FCB_BFC3:FCB_BFC3 + 768] = fc3_b
    fcbuf[:, FCB_ID8:FCB_ID8 + 8] = np.eye(8, dtype=np.float32)

    simple_head = bool(
        np.all(np.asarray(params["gn1_g"]) == 1) and np.all(np.asarray(params["gn1_b"]) == 0)
        and np.all(np.asarray(params["gn2_g"]) == 1) and np.all(np.asarray(params["gn2_b"]) == 0)
        and np.all(fc1_b == 0) and np.all(fc2_b == 0) and np.all(fc3_b == 0)
    )
    ncb = _kernel_b(k2, simple_head)
    in_maps_b = []
    for core in range(NCORES):
        csl = slice(core * CPC, (core + 1) * CPC)
        if k2 == 1:
            g2c = np.ascontiguousarray(g2[csl].transpose(1, 0, 2).reshape(67, CPC * 128))
            n2c = np.ascontiguousarray(new2[csl].transpose(2, 0, 1).reshape(3, CPC * 128))
        else:
            g2c = g2[csl]
            n2c = new2t[csl]
        in_maps_b.append({
            "g2": g2c,
            "new2t": n2c,
            "wb": wbuf,
            "fcb": fcbuf,
        })
    res_b = run_bass_kernel_spmd(ncb, in_maps_b, core_ids=list(range(NCORES)))
    outs = np.stack([r["out"] for r in res_b.results])           # [8,8,768]
    return outs.reshape(B, 768).reshape(B, 3, 256).astype(np.float32)
